# revision 1
# baseline (speedup 1.0000x reference)
"""ProbAttention (Informer-style ProbSparse attention) on 8 Trainium2 cores.

Strategy (per spec sharding hint): pure data parallelism over the 32 (b, h)
pairs -> 4 pairs per NeuronCore, no communication.

Per (b, h) pair, on device:
  1. QK_full = Q @ K^T on PE as three bf16 matmuls per tile
     (Qhi*Khi + Qlo*Khi + Qhi*Klo, hi/lo split on host) -- f32-grade
     accuracy at bf16 speed (validated: max err 4.8e-4, selection-exact).
  2. M[t] = max_s QK[t, idx[t,s]] - (1/T) sum_s QK[t, idx[t,s]]
     against host-built masks: (QK + addmask) -> reduce-max, and
     cnt * (QK + addmask) -> reduce-add (0 * -1e30 = -0.0 is harmless).
  3. top-35 of M via DVE max8/match_replace/max_index rounds, on a shared
     [4 pairs, 512] layout so all pairs pay the serial cost once.
  4. onehot[u, t] = (iota == M_top[u]) via an exact integer-valued f32
     compare; every data-dependent gather/scatter becomes a PE matmul
     with the one-hot matrix (no indirect DMA anywhere):
       scores  = onehotT @ QK_sbuf(f32r)   (row gather)
       update  = softmax(scores/8) @ V     (exp on ACT with fused accum)
       context = onehot^T @ update + (1 - colsum(onehot)) x mean(V)
  5. context -> DRAM, host reassembles [B, T, N, H, D].

Everything is static control flow; Tile handles all semaphores.
"""

import numpy as np
import ml_dtypes

import concourse.bacc as bacc
import concourse.bass as bass
import concourse.mybir as mybir
import concourse.tile as tile
from concourse.bass_utils import run_bass_kernel_spmd
from contextlib import ExitStack

B, T, N, H, D = 4, 512, 4, 8, 64
E = N * D            # 256
U = 35               # sample_k == n_top
NCORES = 8
P = (B * H) // NCORES  # 4 pairs per core
TC = T // 128        # 4 t-chunks
ECH = E // 128       # 2 e-chunks

F32 = mybir.dt.float32
F32R = mybir.dt.float32r
BF16 = mybir.dt.bfloat16
U32 = mybir.dt.uint32
AF = mybir.ActivationFunctionType
ALU = mybir.AluOpType
AX = mybir.AxisListType
NEG = -1.0e30


def _build_program():
    nc = bacc.Bacc("TRN2", target_bir_lowering=False, debug=False)

    # qkp: per pair, partition-major pack of (qh, ql, kh, kl) x (e-chunk)
    qkp_d = nc.dram_tensor("qkp", [P, 128, 4, ECH, T], BF16,
                           kind="ExternalInput")
    v_d = nc.dram_tensor("v", [P, 128, TC, E], F32R, kind="ExternalInput")
    mask_d = nc.dram_tensor("mask", [128, 2, TC, T], BF16,
                            kind="ExternalInput")
    cst_d = nc.dram_tensor("cst", [128, T + 128], F32, kind="ExternalInput")
    cstr_d = nc.dram_tensor("cstr", [128, 129], F32R, kind="ExternalInput")
    out_d = nc.dram_tensor("out", [P, 128, TC, E], F32, kind="ExternalOutput")

    with tile.TileContext(nc) as tc, ExitStack() as ctx:
        const = ctx.enter_context(tc.tile_pool(name="const", bufs=1))
        io_qk = ctx.enter_context(tc.tile_pool(name="io_qk", bufs=2))
        vpool = ctx.enter_context(tc.tile_pool(name="vpool", bufs=P))
        qksb = ctx.enter_context(tc.tile_pool(name="qksb", bufs=2 * P))
        scrp = ctx.enter_context(tc.tile_pool(name="scrp", bufs=2))
        wpool = ctx.enter_context(tc.tile_pool(name="wpool", bufs=2))
        smal = ctx.enter_context(tc.tile_pool(name="smal", bufs=4))
        ohp = ctx.enter_context(tc.tile_pool(name="ohp", bufs=2))
        ohtp = ctx.enter_context(tc.tile_pool(name="ohtp", bufs=8))
        atp = ctx.enter_context(tc.tile_pool(name="atp", bufs=2))
        attp = ctx.enter_context(tc.tile_pool(name="attp", bufs=8))
        ctxp = ctx.enter_context(tc.tile_pool(name="ctxp", bufs=2))
        psp = ctx.enter_context(tc.tile_pool(name="psp", bufs=4, space="PSUM"))

        # ---- constants (masks early; other consts after pair-0 inputs) ----
        masks = const.tile([128, 2, TC, T], BF16, tag="masks")
        cst = const.tile([128, T + 128], F32, tag="cst")
        cstr = const.tile([128, 129], F32R, tag="cstr")
        addm = masks[:, 0]
        cntm = masks[:, 1]
        iota_t = cst[:, 0:T]
        ident = cst[:, T:T + 128]
        identr = cstr[:, 0:128]
        onesr = cstr[:, 128:129]

        mx_cols = const.tile([128, 4 * P], F32, tag="mx")
        sm_cols = const.tile([128, 4 * P], F32, tag="sm")
        m_cols = const.tile([128, 4 * P], F32, tag="mc")
        mT_sb = const.tile([4 * P, 128], F32, tag="mT")
        m_all = const.tile([P, T], F32, tag="mall")
        vals40 = const.tile([P, 40], F32, tag="v40")
        idx40 = const.tile([P, 40], U32, tag="i40")
        idx40f = const.tile([P, 40], F32, tag="i40f")
        idxT = const.tile([40, P], F32, tag="iT")

        vt_all = []
        qk_all = []

        # ============ Phase 1: QK (bf16 hi/lo x3) + M stats per pair ======
        KIND = {"qh": 0, "ql": 1, "kh": 2, "kl": 3}
        for p in range(P):
            qkt = io_qk.tile([128, 4, ECH, T], BF16, tag="qkp", name=f"qkp{p}")
            nc.sync.dma_start(qkt[:], qkp_d[p])
            qk_in = {nm: qkt[:, i] for nm, i in KIND.items()}
            vt = vpool.tile([128, TC, E], F32R, tag="v", name=f"v{p}")
            nc.sync.dma_start(vt[:], v_d[p])
            vt_all.append(vt)
            if p == 0:
                nc.sync.dma_start(masks[:], mask_d[:])
            if p == 1:
                nc.sync.dma_start(cst[:], cst_d[:])
                nc.sync.dma_start(cstr[:], cstr_d[:])

            qks = []
            for half in range(2):
                qk_ps = psp.tile([128, 2 * T], F32, tag="ps",
                                 name=f"qkps{p}_{half}")
                for j in range(2):      # t-chunk within this half
                    tc_i = half * 2 + j
                    sl = slice(j * T, (j + 1) * T)
                    first = True
                    for (lh, rh) in (("qh", "kh"), ("ql", "kh"), ("qh", "kl")):
                        for e in range(ECH):
                            nc.tensor.matmul(
                                qk_ps[:, sl],
                                qk_in[lh][:, e, tc_i * 128:(tc_i + 1) * 128],
                                qk_in[rh][:, e, :],
                                start=first,
                                stop=(lh == "qh" and rh == "kl" and e == 1))
                            first = False
                qk_r = qksb.tile([128, 2 * T], F32R, tag="qksb",
                                 name=f"qkr{p}_{half}")
                nc.scalar.copy(qk_r[:], qk_ps[:])
                qks.append(qk_r)

                col = 4 * p + 2 * half
                qk3 = qk_ps[:].rearrange("p (c k) -> p c k", c=2)
                scr = scrp.tile([128, 2 * T], F32, tag="scr",
                                name=f"scrA{p}_{half}")
                scr3 = scr[:].rearrange("p (c k) -> p c k", c=2)
                nc.vector.tensor_tensor(
                    out=scr3, in0=qk3,
                    in1=addm[:, 2 * half:2 * half + 2, :], op=ALU.add)
                nc.vector.tensor_reduce(
                    out=mx_cols[:, col:col + 2], in_=scr3, axis=AX.X,
                    op=ALU.max)
                scr2 = scrp.tile([128, 2 * T], F32, tag="scr",
                                 name=f"scrB{p}_{half}")
                scr23 = scr2[:].rearrange("p (c k) -> p c k", c=2)
                nc.vector.tensor_tensor(
                    out=scr23, in0=scr3,
                    in1=cntm[:, 2 * half:2 * half + 2, :], op=ALU.mult)
                nc.vector.tensor_reduce(
                    out=sm_cols[:, col:col + 2], in_=scr23, axis=AX.X,
                    op=ALU.add)
            qk_all.append(qks)

        # ============ Phase 2: M assembly + shared top-k ==================
        nc.vector.tensor_scalar(out=sm_cols[:], in0=sm_cols[:],
                                scalar1=-1.0 / T, scalar2=None, op0=ALU.mult)
        nc.vector.tensor_tensor(out=m_cols[:], in0=mx_cols[:], in1=sm_cols[:],
                                op=ALU.add)
        mT_ps = psp.tile([4 * P, 128], F32, tag="ps", name="mTps")
        nc.tensor.transpose(mT_ps[:], m_cols[:], ident[:])
        nc.scalar.copy(mT_sb[:], mT_ps[:])
        for p in range(P):
            nc.sync.dma_start(m_all[p:p + 1, :], mT_sb[4 * p:4 * p + 4, :])

        work = m_all
        for r in range(5):
            nc.vector.max(vals40[:, 8 * r:8 * r + 8], work[:])
            nc.vector.max_index(idx40[:, 8 * r:8 * r + 8],
                                vals40[:, 8 * r:8 * r + 8], m_all[:])
            if r < 4:
                nwork = wpool.tile([P, T], F32, tag="work", name=f"work{r}")
                nc.vector.match_replace(nwork[:], vals40[:, 8 * r:8 * r + 8],
                                        work[:], -1.0e38)
                work = nwork

        nc.vector.tensor_copy(idx40f[:], idx40[:])
        idxT_ps = psp.tile([40, P], F32, tag="ps", name="idxTps")
        nc.tensor.transpose(idxT_ps[:], idx40f[:], ident[0:P, 0:P])
        nc.scalar.copy(idxT[:], idxT_ps[:])

        # ============ Phase 3: attention + context per pair ===============
        for p in range(P):
            onehot = ohp.tile([36, T], F32R, tag="oh", name=f"oh{p}")
            nc.vector.tensor_scalar(out=onehot[:], in0=iota_t[0:36, :],
                                    scalar1=idxT[0:36, p:p + 1], scalar2=None,
                                    op0=ALU.is_equal)
            ohT_ps = psp.tile([128, TC, 36], F32R, tag="ps", name=f"ohTps{p}")
            for c in range(TC):
                nc.tensor.transpose(ohT_ps[:, c, :],
                                    onehot[0:36, c * 128:(c + 1) * 128],
                                    identr[0:36, 0:36])
            ohT_sb = ohtp.tile([128, TC, 36], F32R, tag="ohT", name=f"ohT{p}")
            nc.scalar.copy(ohT_sb[:], ohT_ps[:])
            ohT = [ohT_sb[:, c, :] for c in range(TC)]

            colsum_ps = psp.tile([1, T], F32, tag="ps", name=f"cs{p}")
            nc.tensor.matmul(colsum_ps[:], onesr[0:U, 0:1], onehot[0:U, :],
                             start=True, stop=True)
            notsel = smal.tile([1, T], F32R, tag="ns", name=f"ns{p}")
            nc.scalar.activation(notsel[:], colsum_ps[:], AF.Copy,
                                 bias=1.0, scale=-1.0)

            scores_ps = psp.tile([36, T], F32, tag="ps", name=f"sc{p}")
            for c in range(TC):
                nc.tensor.matmul(
                    scores_ps[:], ohT[c],
                    qk_all[p][c // 2][:, (c % 2) * T:(c % 2 + 1) * T],
                    start=(c == 0), stop=(c == TC - 1))

            attn = atp.tile([36, T], F32, tag="attn", name=f"attn{p}")
            sumexp = smal.tile([36, 1], F32, tag="sx", name=f"sx{p}")
            nc.scalar.activation(attn[:], scores_ps[:], AF.Exp,
                                 bias=0.0, scale=1.0 / np.sqrt(D),
                                 accum_out=sumexp[:])
            recip = smal.tile([36, 1], F32, tag="rc", name=f"rc{p}")
            nc.vector.reciprocal(recip[:], sumexp[:])

            aT_ps = psp.tile([128, TC, 36], F32, tag="ps", name=f"aTps{p}")
            for c in range(TC):
                nc.tensor.transpose(aT_ps[:, c, :],
                                    attn[:, c * 128:(c + 1) * 128],
                                    ident[0:36, 0:36])
            aT_sb = attp.tile([128, TC, 36], F32R, tag="aT", name=f"aT{p}")
            nc.scalar.copy(aT_sb[:], aT_ps[:])
            upd_ps = psp.tile([36, E], F32, tag="ps", name=f"upd{p}")
            for c in range(TC):
                nc.tensor.matmul(upd_ps[:], aT_sb[:, c, :], vt_all[p][:, c, :],
                                 start=(c == 0), stop=(c == TC - 1))
            upd_sb = smal.tile([36, E], F32R, tag="upd", name=f"updsb{p}")
            nc.scalar.activation(upd_sb[:], upd_ps[:], AF.Copy,
                                 bias=0.0, scale=recip[0:36, 0:1])

            vs_ps = psp.tile([1, E], F32, tag="ps", name=f"vs{p}")
            for c in range(TC):
                nc.tensor.matmul(vs_ps[:], onesr[:, 0:1], vt_all[p][:, c, :],
                                 start=(c == 0), stop=(c == TC - 1))
            meanv = smal.tile([1, E], F32R, tag="mv", name=f"mv{p}")
            nc.scalar.activation(meanv[:], vs_ps[:], AF.Copy,
                                 bias=0.0, scale=1.0 / T)

            ctx_sb = ctxp.tile([128, TC, E], F32, tag="ctx", name=f"ctxsb{p}")
            for half in range(2):
                ctx_ps = psp.tile([128, 2, E], F32, tag="ps",
                                  name=f"cx{p}_{half}")
                for j in range(2):
                    c = half * 2 + j
                    nc.tensor.matmul(ctx_ps[:, j, :],
                                     onehot[0:U, c * 128:(c + 1) * 128],
                                     upd_sb[0:U, :], start=True, stop=False)
                    nc.tensor.matmul(ctx_ps[:, j, :],
                                     notsel[0:1, c * 128:(c + 1) * 128],
                                     meanv[:], start=False, stop=True)
                nc.scalar.copy(ctx_sb[:, half * 2:half * 2 + 2, :], ctx_ps[:])
            nc.sync.dma_start(out_d[p], ctx_sb[:])

    nc.finalize()
    return nc


def _round_f32r(x):
    u = np.ascontiguousarray(x, dtype=np.float32).view(np.uint32)
    u = (u + 0x800) & np.uint32(0xFFFFF000)
    return u.view(np.float32)


def _host_prep(queries, keys, values, index_sample):
    q = np.ascontiguousarray(np.asarray(queries, dtype=np.float32))
    k = np.ascontiguousarray(np.asarray(keys, dtype=np.float32))
    v = np.ascontiguousarray(np.asarray(values, dtype=np.float32))
    idx = np.asarray(index_sample).astype(np.int64)

    def merge(x):  # [B,T,N,H,D] -> [B*H, T, E]
        return x.transpose(0, 3, 1, 2, 4).reshape(B, H, T, E).reshape(B * H, T, E)

    qm, km, vm = merge(q), merge(k), merge(v)
    qtm = np.ascontiguousarray(qm.transpose(0, 2, 1))  # [BH, E, T]
    ktm = np.ascontiguousarray(km.transpose(0, 2, 1))

    bf = ml_dtypes.bfloat16
    qh = qtm.astype(bf)
    ql = (qtm - qh.astype(np.float32)).astype(bf)
    kh = ktm.astype(bf)
    kl = (ktm - kh.astype(np.float32)).astype(bf)
    # pack (kind, e-chunk) partition-major: [BH, 128, 4, ECH, T]
    qkp = np.stack([qh, ql, kh, kl], axis=1)          # [BH, 4, E, T]
    qkp = qkp.reshape(B * H, 4, ECH, 128, T).transpose(0, 3, 1, 2, 4)
    qkp = np.ascontiguousarray(qkp)
    # v packed [BH, 128, TC, E]: row (p, c) holds v row t = 128*c + p
    vp = _round_f32r(vm).reshape(B * H, TC, 128, E).transpose(0, 2, 1, 3)
    vp = np.ascontiguousarray(vp)

    cnt = np.zeros((T, T), np.float32)
    np.add.at(cnt, (np.arange(T)[:, None], idx), 1.0)
    addm_full = np.where(cnt > 0, 0.0, NEG).astype(np.float32)
    # pack [T, T] -> [128, TC, T]: row (p, c) holds mask row t = 128*c + p
    pack = lambda m: m.reshape(TC, 128, T).transpose(1, 0, 2)
    mask = np.ascontiguousarray(
        np.stack([pack(addm_full), pack(cnt)], axis=1)).astype(bf)
    iota = np.broadcast_to(np.arange(T, dtype=np.float32), (128, T))
    ident = np.eye(128, dtype=np.float32)
    cst = np.ascontiguousarray(np.concatenate([iota, ident], axis=1))
    cstr = np.ascontiguousarray(np.concatenate(
        [ident, np.ones((128, 1), np.float32)], axis=1))

    in_maps = []
    for c in range(NCORES):
        sl = slice(c * P, (c + 1) * P)
        in_maps.append({
            "qkp": np.ascontiguousarray(qkp[sl]),
            "v": np.ascontiguousarray(vp[sl]),
            "mask": mask, "cst": cst, "cstr": cstr,
        })
    return in_maps


def _host_post(results):
    ctx_all = np.concatenate([np.asarray(r["out"]) for r in results], axis=0)
    # unpack [BH, 128, TC, E] -> [BH, T, E] (t = 128*c + p)
    ctx_all = ctx_all.transpose(0, 2, 1, 3).reshape(B * H, T, E)
    # [B*H, T, E] -> [B, T, N, H, D]
    out = ctx_all.reshape(B, H, T, N, D).transpose(0, 2, 3, 1, 4)
    return np.ascontiguousarray(out.astype(np.float32))


_RUN_KWARGS = {}


def kernel(queries, keys, values, index_sample):
    in_maps = _host_prep(queries, keys, values, index_sample)
    nc = _build_program()
    res = run_bass_kernel_spmd(nc, in_maps, core_ids=list(range(NCORES)),
                               **_RUN_KWARGS)
    out = _host_post(res.results)
    kernel.last_results = res
    return out

